# revision 26
# baseline (speedup 1.0000x reference)
"""RNNT decoder kernel for TRN2 — 8-core SPMD, T-sharded joint,
parallel-in-time (Jacobi) LSTM replicated on each core.

The 2-layer LSTM recurrence is solved by fixed-point iteration: each
sweep recomputes all 64 steps in parallel (batch N = 64*4 = 256) from
the previous sweep's shifted hidden states.  The map is strongly
contractive here, so K0/K1 sweeps reach well below the accuracy target
(validated offline against the sequential recurrence).

Layouts (feature dims on partitions):
  whhT/wihT  [128, (kc4, 2048)] bf16, gate order i|f|o|g~ (host-permuted)
  eysT       [128, (ec4, u64, b4)] bf16
  X0/X1      [128, (gg2, j8, u64, b4)] bf16; j indexes MC_ORDER[gg]
  H bufs     [128, (kc4, 65, b4)] bf16; slot u+1 = h_u, slot 0 = 0
  C bufs     [128, (kc4, 65, b4)] f32
  gates psum [128, (j8, u64, b4)] f32 per big-group (kc pair)
  hencT      [128, (jc4, b4, t32)] bf16
  hdecJT     [128, (jc4, u64, b4)] bf16
  zT         [128, (jc4, u8, b4, t32)] bf16 per u-block
  out dram   [ub8, oc8, hf2, p128, u4, b4, t32] bf16; host un-permutes
"""

import numpy as np
import ml_dtypes

import concourse.bass as bass
import concourse.mybir as mybir
import concourse.tile as tile
from concourse import bacc
from concourse import bass_utils
from concourse.masks import make_identity

B, T, U, E, H, J, OD, G = 4, 256, 64, 512, 512, 512, 1024, 2048
NCORES = 8
TLOC = T // NCORES          # 32
UBLK = 8
NBLK = U // UBLK            # 8
NS = U * B                  # 256, batched sweep width
SLOT = U + 1                # 65 u-slots (slot 0 = zeros)
K0, K1 = 10, 10             # Jacobi sweeps per layer
F32 = mybir.dt.float32
BF16 = mybir.dt.bfloat16
I32 = mybir.dt.int32
AF = mybir.ActivationFunctionType
BF = ml_dtypes.bfloat16

# big-group gg covers kc pair (2gg, 2gg+1); position j in the psum tile
# holds gate chunk MC_ORDER[gg][j]; order = i,i,f,f,o,o,g~,g~
MC_ORDER = [[0, 1, 4, 5, 8, 9, 12, 13], [2, 3, 6, 7, 10, 11, 14, 15]]

_CACHE = {}


def _xproj(nc, PS, wihT, rhs_kc, bT, Xout):
    """X = (rhs.T @ wih).T + b -> [128, (gg2, j8, 256)] bf16."""
    for gg in range(2):
        ps = PS.tile([128, 8 * NS], F32, tag="gates")
        for j in range(8):
            mc = MC_ORDER[gg][j]
            for kc in range(4):
                nc.tensor.matmul(
                    ps[:, j * NS:(j + 1) * NS],
                    lhsT=wihT[:, kc * G + mc * 128: kc * G + (mc + 1) * 128],
                    rhs=rhs_kc[kc],
                    start=(kc == 0), stop=(kc == 3))
        for j in range(8):
            mc = MC_ORDER[gg][j]
            nc.scalar.add(
                Xout[:, (gg * 8 + j) * NS:(gg * 8 + j + 1) * NS],
                ps[:, j * NS:(j + 1) * NS], bT[:, mc:mc + 1])


def _sweep_layer(nc, P, WK, PS, X, whhT, Hb, Cb, nsweeps, ident, ltag):
    """Jacobi sweeps for one LSTM layer. Returns index of final H buffer."""
    Hv = [h[:].rearrange("p (kc s b) -> p kc s b", kc=4, s=SLOT) for h in Hb]
    Cv = [c[:].rearrange("p (kc s b) -> p kc s b", kc=4, s=SLOT) for c in Cb]
    for s in range(nsweeps):
        rd, wr = s % 2, (s + 1) % 2
        # exact-prefix: h_u for u <= s-1 is already exact in both buffers,
        # so sweep s only recomputes u >= um (width w columns of B each).
        um = max(0, s - 1)
        off, w = um * B, (U - um) * B
        pss = [None, None]
        if s > 0:
            # Emit all matmuls before any consume: X copies for both
            # big-groups first (no H dep), then kc-major per group so the PE
            # queue holds maximal ready work at the sweep boundary (copies
            # and kc 0/1 only need the previous sweep's first kc pair).
            # start=True lazily zeroes the whole 2KB bank (j pair), so one
            # bank-wide copy both starts and fully overwrites it.  Group
            # bookkeeping can't express this, hence skip_group_check.
            for gg in range(2):
                pss[gg] = PS.tile([128, 8 * NS], F32, tag="gates",
                                  name=f"gates{gg}")
                for j in range(0, 8, 2):
                    nc.tensor.matmul(
                        pss[gg][:, j * NS:(j + 2) * NS], lhsT=ident[:],
                        rhs=X[:, (gg * 8 + j) * NS:(gg * 8 + j + 2) * NS],
                        start=True, stop=False, skip_group_check=True)
            for gg in range(2):
                for kc in range(4):
                    for j in range(8):
                        mc = MC_ORDER[gg][j]
                        nc.tensor.matmul(
                            pss[gg][:, j * NS + off:(j + 1) * NS],
                            lhsT=whhT[:, kc * G + mc * 128: kc * G + (mc + 1) * 128],
                            rhs=Hb[rd][:, kc * SLOT * B + off: kc * SLOT * B + NS],
                            start=False, stop=(kc == 3), skip_group_check=True)
        for gg in range(2):
            a = 2 * gg
            if s == 0:
                gv = X[:].rearrange("p (c u b) -> p c u b", c=16, u=U)[
                    :, gg * 8:(gg + 1) * 8, um:U, :]
            else:
                gv = pss[gg][:].rearrange("p (c u b) -> p c u b", c=8, u=U)[
                    :, :, um:U, :]
            # i,f sigmoid first: it gates the h critical path; o hides later
            sig = WK.tile([128, 6 * NS], BF16, tag=f"sig{ltag}")
            sigv = sig[:].rearrange("p (c u b) -> p c u b", c=6, u=U)[
                :, :, um:U, :]
            nc.scalar.activation(sigv[:, 0:4], gv[:, 0:4], AF.Sigmoid)
            tg = WK.tile([128, 2 * NS], BF16, tag=f"tg{ltag}")
            tgv = tg[:].rearrange("p (c u b) -> p c u b", c=2, u=U)[
                :, :, um:U, :]
            nc.scalar.activation(tgv, gv[:, 6:8], AF.Tanh)
            nc.scalar.activation(sigv[:, 4:6], gv[:, 4:6], AF.Sigmoid)
            cprev = Cv[rd][:, a:a + 2, um:U, :]
            cnew = Cv[wr][:, a:a + 2, um + 1:SLOT, :]
            t2 = WK.tile([128, 2 * NS], F32, tag=f"t2{ltag}")
            t2v = t2[:].rearrange("p (k u b) -> p k u b", k=2, u=U)[
                :, :, um:U, :]
            nc.vector.tensor_mul(t2v, sigv[:, 0:2], tgv)
            if s == 0:
                nc.vector.tensor_copy(cnew, t2v)
            else:
                t1 = WK.tile([128, 2 * NS], F32, tag=f"t1{ltag}")
                t1v = t1[:].rearrange("p (k u b) -> p k u b", k=2, u=U)[
                    :, :, um:U, :]
                nc.vector.tensor_mul(t1v, sigv[:, 2:4], cprev)
                nc.vector.tensor_add(cnew, t1v, t2v)
            tc = WK.tile([128, 2 * NS], BF16, tag=f"tc{ltag}")
            tcv = tc[:].rearrange("p (k u b) -> p k u b", k=2, u=U)[
                :, :, um:U, :]
            nc.scalar.activation(tcv, cnew, AF.Tanh)
            nc.vector.tensor_mul(Hv[wr][:, a:a + 2, um + 1:SLOT, :],
                                 sigv[:, 4:6], tcv)
    return nsweeps % 2


def _build():
    nc = bacc.Bacc("TRN2", target_bir_lowering=False, debug=False,
                   enable_asserts=False, num_devices=NCORES)
    hs = nc.dram_tensor("hs", [B, TLOC, E], BF16, kind="ExternalInput").ap()
    emb = nc.dram_tensor("emb", [1024, E], BF16, kind="ExternalInput").ap()
    idx = nc.dram_tensor("idx", [B * U], I32, kind="ExternalInput").ap()
    whh0 = nc.dram_tensor("whh0", [H, G], BF16, kind="ExternalInput").ap()
    wih0 = nc.dram_tensor("wih0", [E, G], BF16, kind="ExternalInput").ap()
    whh1 = nc.dram_tensor("whh1", [H, G], BF16, kind="ExternalInput").ap()
    wih1 = nc.dram_tensor("wih1", [H, G], BF16, kind="ExternalInput").ap()
    wenc = nc.dram_tensor("wenc", [E, J], BF16, kind="ExternalInput").ap()
    wdec = nc.dram_tensor("wdec", [H, J], BF16, kind="ExternalInput").ap()
    wout = nc.dram_tensor("wout", [J, OD], BF16, kind="ExternalInput").ap()
    b0 = nc.dram_tensor("b0", [128, 16], F32, kind="ExternalInput").ap()
    b1 = nc.dram_tensor("b1", [128, 16], F32, kind="ExternalInput").ap()
    benc = nc.dram_tensor("benc", [128, 4], F32, kind="ExternalInput").ap()
    bout = nc.dram_tensor("bout", [128, 8], F32, kind="ExternalInput").ap()
    # device-native order: [ub, oc, hf, p, u4, b, t]; host un-permutes.
    yout = nc.dram_tensor("out", [NBLK, 8, 2, 128, UBLK // 2, B, TLOC], BF16,
                          kind="ExternalOutput").ap()

    from contextlib import ExitStack
    with tile.TileContext(nc) as tc, ExitStack() as ctx:
        P = ctx.enter_context(tc.tile_pool(name="persist", bufs=1))
        WK = ctx.enter_context(tc.tile_pool(name="work", bufs=3))
        DBL = ctx.enter_context(tc.tile_pool(name="dbl", bufs=2))
        Z4 = ctx.enter_context(tc.tile_pool(name="z4", bufs=4))
        Z8 = ctx.enter_context(tc.tile_pool(name="z8", bufs=8))

        # ---- tiny inputs first (they gate the gather/transpose chain) ----
        idx_sb = P.tile([128, 2], I32, tag="idx")
        for r in range(2):
            nc.sync.dma_start(idx_sb[:, r:r + 1],
                              idx[r * 128:(r + 1) * 128].unsqueeze(1))
        b0T = P.tile([128, 16], F32, tag="b0T")
        nc.sync.dma_start(b0T[:], b0)
        b1T = P.tile([128, 16], F32, tag="b1T")
        nc.sync.dma_start(b1T[:], b1)
        bencT = P.tile([128, 4], F32, tag="bencT")
        nc.sync.dma_start(bencT[:], benc)
        boutT = P.tile([128, 8], F32, tag="boutT")
        nc.sync.dma_start(boutT[:], bout)
        hs_sb = P.tile([128, E], BF16, tag="hs_sb")
        for b in range(B):
            nc.sync.dma_start(hs_sb[b * TLOC:(b + 1) * TLOC, :], hs[b])

        # ---- weight loads, spread across engine DGE queues so they run in
        # parallel (a single queue serializes ~40us of weight traffic) ----
        wih0T = P.tile([128, 4 * G], BF16, tag="wih0T")
        nc.sync.dma_start(wih0T[:].rearrange("p (kc j) -> p kc j", kc=4),
                          wih0.rearrange("(kc p) j -> p kc j", p=128))
        whh0T = P.tile([128, 4 * G], BF16, tag="whh0T")
        nc.scalar.dma_start(whh0T[:].rearrange("p (kc j) -> p kc j", kc=4),
                            whh0.rearrange("(kc p) j -> p kc j", p=128))
        whh1T = P.tile([128, 4 * G], BF16, tag="whh1T")
        nc.gpsimd.dma_start(whh1T[:].rearrange("p (kc j) -> p kc j", kc=4),
                            whh1.rearrange("(kc p) j -> p kc j", p=128))
        wih1T = P.tile([128, 4 * G], BF16, tag="wih1T")
        nc.sync.dma_start(wih1T[:].rearrange("p (kc j) -> p kc j", kc=4),
                          wih1.rearrange("(kc p) j -> p kc j", p=128))
        wencT = P.tile([128, 4 * J], BF16, tag="wencT")
        nc.scalar.dma_start(wencT[:].rearrange("p (kc j) -> p kc j", kc=4),
                            wenc.rearrange("(kc p) j -> p kc j", p=128))
        wdecT = P.tile([128, 4 * J], BF16, tag="wdecT")
        nc.scalar.dma_start(wdecT[:].rearrange("p (kc j) -> p kc j", kc=4),
                            wdec.rearrange("(kc p) j -> p kc j", p=128))
        woutT = P.tile([128, 4 * OD], BF16, tag="woutT")
        nc.gpsimd.dma_start(woutT[:].rearrange("p (kc j) -> p kc j", kc=4),
                            wout.rearrange("(kc p) j -> p kc j", p=128))

        ident = P.tile([128, 128], BF16, tag="ident")
        make_identity(nc, ident[:])

        # ---- prologue: gather/transpose (own psum pool scope) ----
        eysT = P.tile([128, 4 * NS], BF16, tag="eysT")
        hencT = P.tile([128, 4 * B * TLOC], BF16, tag="hencT")
        with tc.tile_pool(name="ps_pro", bufs=2, space="PSUM") as PSP:
            for r in range(2):
                eys_sb = P.tile([128, E], BF16, tag=f"eys{r}")
                nc.gpsimd.indirect_dma_start(
                    out=eys_sb[:], out_offset=None, in_=emb,
                    in_offset=bass.IndirectOffsetOnAxis(ap=idx_sb[:, r:r + 1], axis=0))
                for ec in range(4):
                    pst = PSP.tile([128, 128], BF16, tag="tp")
                    nc.tensor.transpose(out=pst[:], in_=eys_sb[:, ec * 128:(ec + 1) * 128],
                                        identity=ident[:])
                    nc.vector.tensor_copy(
                        eysT[:, ec * NS + r * 128: ec * NS + r * 128 + 128], pst[:])

            # hs slice -> hsT [128, (ec, b, t)]
            hsT = P.tile([128, 4 * 128], BF16, tag="hsT")
            for ec in range(4):
                pst = PSP.tile([128, 128], BF16, tag="tp")
                nc.tensor.transpose(out=pst[:], in_=hs_sb[:, ec * 128:(ec + 1) * 128],
                                    identity=ident[:])
                nc.vector.tensor_copy(hsT[:, ec * 128:(ec + 1) * 128], pst[:])

            # henc -> hencT [128, (jc, b, t)] bf16
            for jc in range(4):
                ps = PSP.tile([128, 128], F32, tag="henc")
                for kc in range(4):
                    nc.tensor.matmul(
                        ps[:], lhsT=wencT[:, kc * J + jc * 128: kc * J + jc * 128 + 128],
                        rhs=hsT[:, kc * 128:(kc + 1) * 128],
                        start=(kc == 0), stop=(kc == 3))
                nc.vector.tensor_scalar_add(hencT[:, jc * 128:(jc + 1) * 128], ps[:],
                                            bencT[:, jc:jc + 1])

        # ---- LSTM phases (big psum pool scope) ----
        hdecJT = P.tile([128, 4 * NS], BF16, tag="hdecJT")
        with tc.tile_pool(name="ps_lstm", bufs=2, space="PSUM") as PSL:
            X0 = P.tile([128, 16 * NS], BF16, tag="X")
            _xproj(nc, PSL, wih0T,
                   [eysT[:, kc * NS:(kc + 1) * NS] for kc in range(4)], b0T, X0)

            H0a = P.tile([128, 4 * SLOT * B], BF16, tag="H0a")
            H0b = P.tile([128, 4 * SLOT * B], BF16, tag="H0b")
            C0a = P.tile([128, 4 * SLOT * B], F32, tag="Ca")
            C0b = P.tile([128, 4 * SLOT * B], F32, tag="Cb")
            for t_ in (H0a, H0b, C0a, C0b):
                nc.vector.memset(t_[:], 0.0)
            f0 = _sweep_layer(nc, P, WK, PSL, X0, whh0T, [H0a, H0b],
                              [C0a, C0b], K0, ident, "0")
            H0f = [H0a, H0b][f0]

            X1 = P.tile([128, 16 * NS], BF16, tag="X")
            _xproj(nc, PSL, wih1T,
                   [H0f[:, kc * SLOT * B + B: kc * SLOT * B + B + NS]
                    for kc in range(4)], b1T, X1)

            H1a = P.tile([128, 4 * SLOT * B], BF16, tag="H1a")
            H1b = P.tile([128, 4 * SLOT * B], BF16, tag="H1b")
            C1a = P.tile([128, 4 * SLOT * B], F32, tag="Ca")
            C1b = P.tile([128, 4 * SLOT * B], F32, tag="Cb")
            for t_ in (H1a, H1b, C1a, C1b):
                nc.vector.memset(t_[:], 0.0)
            f1 = _sweep_layer(nc, P, WK, PSL, X1, whh1T, [H1a, H1b],
                              [C1a, C1b], K1, ident, "1")
            H1f = [H1a, H1b][f1]

            # hdecJ = h_dec @ W_dec.T -> hdecJT [128, (jc, u, b)] bf16
            ps = PSL.tile([128, 8 * NS], F32, tag="gates")
            for jc in range(4):
                for kc in range(4):
                    nc.tensor.matmul(
                        ps[:, jc * NS:(jc + 1) * NS],
                        lhsT=wdecT[:, kc * J + jc * 128: kc * J + jc * 128 + 128],
                        rhs=H1f[:, kc * SLOT * B + B: kc * SLOT * B + B + NS],
                        start=(kc == 0), stop=(kc == 3))
            nc.vector.tensor_copy(hdecJT[:], ps[:, 0:4 * NS])

        # ---- joint, per u-block (own psum pool) ----
        outv = yout.rearrange("ub oc hf p u b t -> oc ub hf p u b t")
        with tc.tile_pool(name="ps_joint", bufs=6, space="PSUM") as PSJ:
            for ub in range(NBLK):
                zT = DBL.tile([128, 4 * UBLK * B * TLOC], BF16, tag="zT")
                for jc in range(4):
                    zin = Z4.tile([128, UBLK * B * TLOC], BF16, tag="zin")
                    henc_bc = (hencT[:, jc * 128:(jc + 1) * 128]
                               .rearrange("p (b t) -> p b t", b=B)
                               .unsqueeze(1).to_broadcast([128, UBLK, B, TLOC]))
                    hdec_bc = (hdecJT[:, jc * NS + ub * UBLK * B: jc * NS + (ub + 1) * UBLK * B]
                               .rearrange("p (u b) -> p u b", u=UBLK)
                               .unsqueeze(3).to_broadcast([128, UBLK, B, TLOC]))
                    nc.vector.tensor_add(
                        zin[:].rearrange("p (u b t) -> p u b t", u=UBLK, b=B),
                        henc_bc, hdec_bc)
                    nc.scalar.activation(zT[:, jc * 1024:(jc + 1) * 1024], zin[:],
                                         AF.Tanh)
                for oc in range(8):
                    for hf in range(2):
                        ps = PSJ.tile([128, 512], F32, tag="out")
                        for jc in range(4):
                            nc.tensor.matmul(
                                ps[:],
                                lhsT=woutT[:, jc * OD + oc * 128: jc * OD + oc * 128 + 128],
                                rhs=zT[:, jc * 1024 + hf * 512: jc * 1024 + hf * 512 + 512],
                                start=(jc == 0), stop=(jc == 3))
                        zout = Z8.tile([128, 512], BF16, tag="zout")
                        if (oc * 2 + hf) % 2 == 0:
                            nc.vector.tensor_scalar_add(zout[:], ps[:],
                                                        boutT[:, oc:oc + 1])
                        else:
                            nc.scalar.add(zout[:], ps[:], boutT[:, oc:oc + 1])
                        deng = nc.sync if (oc * 2 + hf) % 2 == 0 else nc.gpsimd
                        deng.dma_start(
                            outv[oc, ub, hf],
                            zout[:].rearrange("p (u b t) -> p u b t", u=UBLK // 2, b=B))
    nc.compile()
    return nc


def _get_nc():
    if "nc" not in _CACHE:
        _CACHE["nc"] = _build()
    return _CACHE["nc"]


# torch gate order (i, f, g, o) -> device order (i, f, o, g~)
_PERM = np.concatenate([np.arange(0, 512), np.arange(512, 1024),
                        np.arange(1536, 2048), np.arange(1024, 1536)])


def _prep_w(w):
    """[2048, 512] f32 -> [512, 2048] bf16, gate-permuted."""
    return np.ascontiguousarray(np.asarray(w, np.float32)[_PERM].T).astype(BF)


def _prep_b(b):
    """[2048] f32 (permuted) -> [128, 16] p-major (value for gate mc*128+p)."""
    return np.ascontiguousarray(b.reshape(16, 128).T)


def _make_in_maps(inputs):
    hs_pad = np.asarray(inputs["hs_pad"], np.float32)
    ys_pad = np.asarray(inputs["ys_pad"])
    embed = np.asarray(inputs["embed"], np.float32)

    ys_in = np.concatenate([np.zeros((B, 1), ys_pad.dtype), ys_pad], axis=1)
    idx = np.ascontiguousarray(ys_in.T).reshape(-1).astype(np.int32)  # u-major

    common = {
        "emb": embed.astype(BF),
        "idx": idx,
        "whh0": _prep_w(inputs["W_hh0"]),
        "wih0": _prep_w(inputs["W_ih0"]),
        "whh1": _prep_w(inputs["W_hh1"]),
        "wih1": _prep_w(inputs["W_ih1"]),
        "wenc": np.ascontiguousarray(np.asarray(inputs["W_enc"], np.float32).T).astype(BF),
        "wdec": np.ascontiguousarray(np.asarray(inputs["W_dec"], np.float32).T).astype(BF),
        "wout": np.ascontiguousarray(np.asarray(inputs["W_out"], np.float32).T).astype(BF),
        "b0": _prep_b((np.asarray(inputs["b_ih0"], np.float32)
                       + np.asarray(inputs["b_hh0"], np.float32))[_PERM]),
        "b1": _prep_b((np.asarray(inputs["b_ih1"], np.float32)
                       + np.asarray(inputs["b_hh1"], np.float32))[_PERM]),
        "benc": np.ascontiguousarray(
            np.asarray(inputs["b_enc"], np.float32).reshape(4, 128).T),
        "bout": np.ascontiguousarray(
            np.asarray(inputs["b_out"], np.float32).reshape(8, 128).T),
    }
    in_maps = []
    for c in range(NCORES):
        m = dict(common)
        m["hs"] = np.ascontiguousarray(
            hs_pad[:, c * TLOC:(c + 1) * TLOC, :]).astype(BF)
        in_maps.append(m)
    return in_maps


def _assemble_core_output(o):
    # [ub, oc, hf, p, u4, b, t] -> (B, TLOC, U=ub*8+hf*4+u4, OD=oc*128+p)
    o = np.asarray(o).reshape(NBLK, 8, 2, 128, UBLK // 2, B, TLOC)
    o = np.transpose(o, (5, 6, 0, 2, 4, 1, 3))
    return np.ascontiguousarray(o).reshape(B, TLOC, U, OD).astype(np.float32)


def kernel(**inputs):
    nc = _get_nc()
    in_maps = _make_in_maps(inputs)
    _CACHE["in_maps"] = in_maps
    res = bass_utils.run_bass_kernel_spmd(nc, in_maps, core_ids=list(range(NCORES)))
    outs = [_assemble_core_output(r["out"]) for r in res.results]
    return np.concatenate(outs, axis=1).astype(np.float32)


# revision 27
# speedup vs baseline: 1.0005x; 1.0005x over previous
"""RNNT decoder kernel for TRN2 — 8-core SPMD, T-sharded joint,
parallel-in-time (Jacobi) LSTM replicated on each core.

The 2-layer LSTM recurrence is solved by fixed-point iteration: each
sweep recomputes all 64 steps in parallel (batch N = 64*4 = 256) from
the previous sweep's shifted hidden states.  The map is strongly
contractive here, so K0/K1 sweeps reach well below the accuracy target
(validated offline against the sequential recurrence).

Layouts (feature dims on partitions):
  whhT/wihT  [128, (kc4, 2048)] bf16, gate order i|f|o|g~ (host-permuted)
  eysT       [128, (ec4, u64, b4)] bf16
  X0/X1      [128, (gg2, j8, u64, b4)] bf16; j indexes MC_ORDER[gg]
  H bufs     [128, (kc4, 65, b4)] bf16; slot u+1 = h_u, slot 0 = 0
  C bufs     [128, (kc4, 65, b4)] f32
  gates psum [128, (j8, u64, b4)] f32 per big-group (kc pair)
  hencT      [128, (jc4, b4, t32)] bf16
  hdecJT     [128, (jc4, u64, b4)] bf16
  zT         [128, (jc4, u8, b4, t32)] bf16 per u-block
  out dram   [ub8, oc8, hf2, p128, u4, b4, t32] bf16; host un-permutes
"""

import numpy as np
import ml_dtypes

import concourse.bass as bass
import concourse.mybir as mybir
import concourse.tile as tile
from concourse import bacc
from concourse import bass_utils
from concourse.masks import make_identity

B, T, U, E, H, J, OD, G = 4, 256, 64, 512, 512, 512, 1024, 2048
NCORES = 8
TLOC = T // NCORES          # 32
UBLK = 8
NBLK = U // UBLK            # 8
NS = U * B                  # 256, batched sweep width
SLOT = U + 1                # 65 u-slots (slot 0 = zeros)
K0, K1 = 10, 10             # Jacobi sweeps per layer
F32 = mybir.dt.float32
BF16 = mybir.dt.bfloat16
I32 = mybir.dt.int32
AF = mybir.ActivationFunctionType
BF = ml_dtypes.bfloat16

# big-group gg covers kc pair (2gg, 2gg+1); position j in the psum tile
# holds gate chunk MC_ORDER[gg][j]; order = i,i,f,f,o,o,g~,g~
MC_ORDER = [[0, 1, 4, 5, 8, 9, 12, 13], [2, 3, 6, 7, 10, 11, 14, 15]]

_CACHE = {}


def _xproj(nc, PS, wihT, rhs_kc, bT, Xout):
    """X = (rhs.T @ wih).T + b -> [128, (gg2, j8, 256)] bf16."""
    for gg in range(2):
        ps = PS.tile([128, 8 * NS], F32, tag="gates")
        for j in range(8):
            mc = MC_ORDER[gg][j]
            for kc in range(4):
                nc.tensor.matmul(
                    ps[:, j * NS:(j + 1) * NS],
                    lhsT=wihT[:, kc * G + mc * 128: kc * G + (mc + 1) * 128],
                    rhs=rhs_kc[kc],
                    start=(kc == 0), stop=(kc == 3))
        for j in range(8):
            mc = MC_ORDER[gg][j]
            nc.scalar.add(
                Xout[:, (gg * 8 + j) * NS:(gg * 8 + j + 1) * NS],
                ps[:, j * NS:(j + 1) * NS], bT[:, mc:mc + 1])


def _sweep_layer(nc, P, WK, PS, X, whhT, Hb, Cb, nsweeps, ident, ltag):
    """Jacobi sweeps for one LSTM layer. Returns index of final H buffer."""
    Hv = [h[:].rearrange("p (kc s b) -> p kc s b", kc=4, s=SLOT) for h in Hb]
    Cv = [c[:].rearrange("p (kc s b) -> p kc s b", kc=4, s=SLOT) for c in Cb]
    for s in range(nsweeps):
        rd, wr = s % 2, (s + 1) % 2
        # exact-prefix: h_u for u <= s-1 is already exact in both buffers,
        # so sweep s only recomputes u >= um (width w columns of B each).
        um = max(0, s - 1)
        off, w = um * B, (U - um) * B
        pss = [None, None]
        if s > 0:
            # Emit all matmuls before any consume: X copies for both
            # big-groups first (no H dep), then kc-major per group so the PE
            # queue holds maximal ready work at the sweep boundary (copies
            # and kc 0/1 only need the previous sweep's first kc pair).
            # start=True lazily zeroes the whole 2KB bank (j pair), so one
            # bank-wide copy both starts and fully overwrites it.  Group
            # bookkeeping can't express this, hence skip_group_check.
            for gg in range(2):
                pss[gg] = PS.tile([128, 8 * NS], F32, tag="gates",
                                  name=f"gates{gg}")
                for j in range(0, 8, 2):
                    nc.tensor.matmul(
                        pss[gg][:, j * NS:(j + 2) * NS], lhsT=ident[:],
                        rhs=X[:, (gg * 8 + j) * NS:(gg * 8 + j + 2) * NS],
                        start=True, stop=False, skip_group_check=True)
            for gg in range(2):
                for kc in range(4):
                    for j in range(8):
                        mc = MC_ORDER[gg][j]
                        nc.tensor.matmul(
                            pss[gg][:, j * NS + off:(j + 1) * NS],
                            lhsT=whhT[:, kc * G + mc * 128: kc * G + (mc + 1) * 128],
                            rhs=Hb[rd][:, kc * SLOT * B + off: kc * SLOT * B + NS],
                            start=False, stop=(kc == 3), skip_group_check=True)
        for gg in range(2):
            a = 2 * gg
            if s == 0:
                gv = X[:].rearrange("p (c u b) -> p c u b", c=16, u=U)[
                    :, gg * 8:(gg + 1) * 8, um:U, :]
            else:
                gv = pss[gg][:].rearrange("p (c u b) -> p c u b", c=8, u=U)[
                    :, :, um:U, :]
            # i,f sigmoid first: it gates the h critical path; o hides later
            sig = WK.tile([128, 6 * NS], BF16, tag=f"sig{ltag}")
            sigv = sig[:].rearrange("p (c u b) -> p c u b", c=6, u=U)[
                :, :, um:U, :]
            nc.scalar.activation(sigv[:, 0:4], gv[:, 0:4], AF.Sigmoid)
            tg = WK.tile([128, 2 * NS], BF16, tag=f"tg{ltag}")
            tgv = tg[:].rearrange("p (c u b) -> p c u b", c=2, u=U)[
                :, :, um:U, :]
            nc.scalar.activation(tgv, gv[:, 6:8], AF.Tanh)
            nc.scalar.activation(sigv[:, 4:6], gv[:, 4:6], AF.Sigmoid)
            cprev = Cv[rd][:, a:a + 2, um:U, :]
            cnew = Cv[wr][:, a:a + 2, um + 1:SLOT, :]
            t2 = WK.tile([128, 2 * NS], F32, tag=f"t2{ltag}")
            t2v = t2[:].rearrange("p (k u b) -> p k u b", k=2, u=U)[
                :, :, um:U, :]
            nc.vector.tensor_mul(t2v, sigv[:, 0:2], tgv)
            if s == 0:
                nc.vector.tensor_copy(cnew, t2v)
            else:
                t1 = WK.tile([128, 2 * NS], F32, tag=f"t1{ltag}")
                t1v = t1[:].rearrange("p (k u b) -> p k u b", k=2, u=U)[
                    :, :, um:U, :]
                nc.vector.tensor_mul(t1v, sigv[:, 2:4], cprev)
                nc.vector.tensor_add(cnew, t1v, t2v)
            tc = WK.tile([128, 2 * NS], BF16, tag=f"tc{ltag}")
            tcv = tc[:].rearrange("p (k u b) -> p k u b", k=2, u=U)[
                :, :, um:U, :]
            nc.scalar.activation(tcv, cnew, AF.Tanh)
            nc.vector.tensor_mul(Hv[wr][:, a:a + 2, um + 1:SLOT, :],
                                 sigv[:, 4:6], tcv)
    return nsweeps % 2


def _build():
    nc = bacc.Bacc("TRN2", target_bir_lowering=False, debug=False,
                   enable_asserts=False, num_devices=NCORES)
    # eysT/hsT come pre-transposed from the host: [p, ec, cols] contiguous
    eyst_in = nc.dram_tensor("eyst", [128, 4, NS], BF16, kind="ExternalInput").ap()
    hst_in = nc.dram_tensor("hst", [128, 4, B * TLOC], BF16, kind="ExternalInput").ap()
    whh0 = nc.dram_tensor("whh0", [H, G], BF16, kind="ExternalInput").ap()
    wih0 = nc.dram_tensor("wih0", [E, G], BF16, kind="ExternalInput").ap()
    whh1 = nc.dram_tensor("whh1", [H, G], BF16, kind="ExternalInput").ap()
    wih1 = nc.dram_tensor("wih1", [H, G], BF16, kind="ExternalInput").ap()
    wenc = nc.dram_tensor("wenc", [E, J], BF16, kind="ExternalInput").ap()
    wdec = nc.dram_tensor("wdec", [H, J], BF16, kind="ExternalInput").ap()
    wout = nc.dram_tensor("wout", [J, OD], BF16, kind="ExternalInput").ap()
    b0 = nc.dram_tensor("b0", [128, 16], F32, kind="ExternalInput").ap()
    b1 = nc.dram_tensor("b1", [128, 16], F32, kind="ExternalInput").ap()
    benc = nc.dram_tensor("benc", [128, 4], F32, kind="ExternalInput").ap()
    bout = nc.dram_tensor("bout", [128, 8], F32, kind="ExternalInput").ap()
    # device-native order: [ub, oc, hf, p, u4, b, t]; host un-permutes.
    yout = nc.dram_tensor("out", [NBLK, 8, 2, 128, UBLK // 2, B, TLOC], BF16,
                          kind="ExternalOutput").ap()

    from contextlib import ExitStack
    with tile.TileContext(nc) as tc, ExitStack() as ctx:
        P = ctx.enter_context(tc.tile_pool(name="persist", bufs=1))
        WK = ctx.enter_context(tc.tile_pool(name="work", bufs=3))
        DBL = ctx.enter_context(tc.tile_pool(name="dbl", bufs=2))
        Z4 = ctx.enter_context(tc.tile_pool(name="z4", bufs=4))
        Z8 = ctx.enter_context(tc.tile_pool(name="z8", bufs=8))

        # ---- activation inputs first: they gate X0 / henc ----
        eysT = P.tile([128, 4 * NS], BF16, tag="eysT")
        nc.sync.dma_start(eysT[:].rearrange("p (ec n) -> p ec n", ec=4), eyst_in)
        hsT = P.tile([128, 4 * 128], BF16, tag="hsT")
        nc.scalar.dma_start(hsT[:].rearrange("p (ec n) -> p ec n", ec=4), hst_in)
        b0T = P.tile([128, 16], F32, tag="b0T")
        nc.sync.dma_start(b0T[:], b0)
        b1T = P.tile([128, 16], F32, tag="b1T")
        nc.sync.dma_start(b1T[:], b1)
        bencT = P.tile([128, 4], F32, tag="bencT")
        nc.gpsimd.dma_start(bencT[:], benc)
        boutT = P.tile([128, 8], F32, tag="boutT")
        nc.gpsimd.dma_start(boutT[:], bout)

        # ---- weight loads, spread across engine DGE queues so they run in
        # parallel (a single queue serializes ~40us of weight traffic) ----
        wih0T = P.tile([128, 4 * G], BF16, tag="wih0T")
        nc.sync.dma_start(wih0T[:].rearrange("p (kc j) -> p kc j", kc=4),
                          wih0.rearrange("(kc p) j -> p kc j", p=128))
        whh0T = P.tile([128, 4 * G], BF16, tag="whh0T")
        nc.scalar.dma_start(whh0T[:].rearrange("p (kc j) -> p kc j", kc=4),
                            whh0.rearrange("(kc p) j -> p kc j", p=128))
        whh1T = P.tile([128, 4 * G], BF16, tag="whh1T")
        nc.gpsimd.dma_start(whh1T[:].rearrange("p (kc j) -> p kc j", kc=4),
                            whh1.rearrange("(kc p) j -> p kc j", p=128))
        wih1T = P.tile([128, 4 * G], BF16, tag="wih1T")
        nc.sync.dma_start(wih1T[:].rearrange("p (kc j) -> p kc j", kc=4),
                          wih1.rearrange("(kc p) j -> p kc j", p=128))
        wencT = P.tile([128, 4 * J], BF16, tag="wencT")
        nc.scalar.dma_start(wencT[:].rearrange("p (kc j) -> p kc j", kc=4),
                            wenc.rearrange("(kc p) j -> p kc j", p=128))
        wdecT = P.tile([128, 4 * J], BF16, tag="wdecT")
        nc.scalar.dma_start(wdecT[:].rearrange("p (kc j) -> p kc j", kc=4),
                            wdec.rearrange("(kc p) j -> p kc j", p=128))
        woutT = P.tile([128, 4 * OD], BF16, tag="woutT")
        nc.gpsimd.dma_start(woutT[:].rearrange("p (kc j) -> p kc j", kc=4),
                            wout.rearrange("(kc p) j -> p kc j", p=128))

        ident = P.tile([128, 128], BF16, tag="ident")
        make_identity(nc, ident[:])

        # ---- prologue: henc (own psum pool scope) ----
        hencT = P.tile([128, 4 * B * TLOC], BF16, tag="hencT")
        with tc.tile_pool(name="ps_pro", bufs=2, space="PSUM") as PSP:
            # henc -> hencT [128, (jc, b, t)] bf16
            for jc in range(4):
                ps = PSP.tile([128, 128], F32, tag="henc")
                for kc in range(4):
                    nc.tensor.matmul(
                        ps[:], lhsT=wencT[:, kc * J + jc * 128: kc * J + jc * 128 + 128],
                        rhs=hsT[:, kc * 128:(kc + 1) * 128],
                        start=(kc == 0), stop=(kc == 3))
                nc.vector.tensor_scalar_add(hencT[:, jc * 128:(jc + 1) * 128], ps[:],
                                            bencT[:, jc:jc + 1])

        # ---- LSTM phases (big psum pool scope) ----
        hdecJT = P.tile([128, 4 * NS], BF16, tag="hdecJT")
        with tc.tile_pool(name="ps_lstm", bufs=2, space="PSUM") as PSL:
            X0 = P.tile([128, 16 * NS], BF16, tag="X")
            _xproj(nc, PSL, wih0T,
                   [eysT[:, kc * NS:(kc + 1) * NS] for kc in range(4)], b0T, X0)

            H0a = P.tile([128, 4 * SLOT * B], BF16, tag="H0a")
            H0b = P.tile([128, 4 * SLOT * B], BF16, tag="H0b")
            C0a = P.tile([128, 4 * SLOT * B], F32, tag="Ca")
            C0b = P.tile([128, 4 * SLOT * B], F32, tag="Cb")
            for t_ in (H0a, H0b, C0a, C0b):
                nc.vector.memset(t_[:], 0.0)
            f0 = _sweep_layer(nc, P, WK, PSL, X0, whh0T, [H0a, H0b],
                              [C0a, C0b], K0, ident, "0")
            H0f = [H0a, H0b][f0]

            X1 = P.tile([128, 16 * NS], BF16, tag="X")
            _xproj(nc, PSL, wih1T,
                   [H0f[:, kc * SLOT * B + B: kc * SLOT * B + B + NS]
                    for kc in range(4)], b1T, X1)

            H1a = P.tile([128, 4 * SLOT * B], BF16, tag="H1a")
            H1b = P.tile([128, 4 * SLOT * B], BF16, tag="H1b")
            C1a = P.tile([128, 4 * SLOT * B], F32, tag="Ca")
            C1b = P.tile([128, 4 * SLOT * B], F32, tag="Cb")
            for t_ in (H1a, H1b, C1a, C1b):
                nc.vector.memset(t_[:], 0.0)
            f1 = _sweep_layer(nc, P, WK, PSL, X1, whh1T, [H1a, H1b],
                              [C1a, C1b], K1, ident, "1")
            H1f = [H1a, H1b][f1]

            # hdecJ = h_dec @ W_dec.T -> hdecJT [128, (jc, u, b)] bf16
            ps = PSL.tile([128, 8 * NS], F32, tag="gates")
            for jc in range(4):
                for kc in range(4):
                    nc.tensor.matmul(
                        ps[:, jc * NS:(jc + 1) * NS],
                        lhsT=wdecT[:, kc * J + jc * 128: kc * J + jc * 128 + 128],
                        rhs=H1f[:, kc * SLOT * B + B: kc * SLOT * B + B + NS],
                        start=(kc == 0), stop=(kc == 3))
            nc.vector.tensor_copy(hdecJT[:], ps[:, 0:4 * NS])

        # ---- joint, per u-block (own psum pool) ----
        outv = yout.rearrange("ub oc hf p u b t -> oc ub hf p u b t")
        with tc.tile_pool(name="ps_joint", bufs=6, space="PSUM") as PSJ:
            for ub in range(NBLK):
                zT = DBL.tile([128, 4 * UBLK * B * TLOC], BF16, tag="zT")
                for jc in range(4):
                    zin = Z4.tile([128, UBLK * B * TLOC], BF16, tag="zin")
                    henc_bc = (hencT[:, jc * 128:(jc + 1) * 128]
                               .rearrange("p (b t) -> p b t", b=B)
                               .unsqueeze(1).to_broadcast([128, UBLK, B, TLOC]))
                    hdec_bc = (hdecJT[:, jc * NS + ub * UBLK * B: jc * NS + (ub + 1) * UBLK * B]
                               .rearrange("p (u b) -> p u b", u=UBLK)
                               .unsqueeze(3).to_broadcast([128, UBLK, B, TLOC]))
                    nc.vector.tensor_add(
                        zin[:].rearrange("p (u b t) -> p u b t", u=UBLK, b=B),
                        henc_bc, hdec_bc)
                    nc.scalar.activation(zT[:, jc * 1024:(jc + 1) * 1024], zin[:],
                                         AF.Tanh)
                for oc in range(8):
                    for hf in range(2):
                        ps = PSJ.tile([128, 512], F32, tag="out")
                        for jc in range(4):
                            nc.tensor.matmul(
                                ps[:],
                                lhsT=woutT[:, jc * OD + oc * 128: jc * OD + oc * 128 + 128],
                                rhs=zT[:, jc * 1024 + hf * 512: jc * 1024 + hf * 512 + 512],
                                start=(jc == 0), stop=(jc == 3))
                        zout = Z8.tile([128, 512], BF16, tag="zout")
                        if (oc * 2 + hf) % 2 == 0:
                            nc.vector.tensor_scalar_add(zout[:], ps[:],
                                                        boutT[:, oc:oc + 1])
                        else:
                            nc.scalar.add(zout[:], ps[:], boutT[:, oc:oc + 1])
                        deng = nc.sync if (oc * 2 + hf) % 2 == 0 else nc.gpsimd
                        deng.dma_start(
                            outv[oc, ub, hf],
                            zout[:].rearrange("p (u b t) -> p u b t", u=UBLK // 2, b=B))
    nc.compile()
    return nc


def _get_nc():
    if "nc" not in _CACHE:
        _CACHE["nc"] = _build()
    return _CACHE["nc"]


# torch gate order (i, f, g, o) -> device order (i, f, o, g~)
_PERM = np.concatenate([np.arange(0, 512), np.arange(512, 1024),
                        np.arange(1536, 2048), np.arange(1024, 1536)])


def _prep_w(w):
    """[2048, 512] f32 -> [512, 2048] bf16, gate-permuted."""
    return np.ascontiguousarray(np.asarray(w, np.float32)[_PERM].T).astype(BF)


def _prep_b(b):
    """[2048] f32 (permuted) -> [128, 16] p-major (value for gate mc*128+p)."""
    return np.ascontiguousarray(b.reshape(16, 128).T)


def _make_in_maps(inputs):
    hs_pad = np.asarray(inputs["hs_pad"], np.float32)
    ys_pad = np.asarray(inputs["ys_pad"])
    embed = np.asarray(inputs["embed"], np.float32)

    ys_in = np.concatenate([np.zeros((B, 1), ys_pad.dtype), ys_pad], axis=1)
    # eysT: embed rows for (u, b) u-major, transposed to [p, ec, (u b)]
    eys = embed[ys_in.T.reshape(-1)]                   # (U*B, E)
    eyst = np.ascontiguousarray(
        eys.T.reshape(4, 128, U * B).transpose(1, 0, 2)).astype(BF)

    common = {
        "eyst": eyst,
        "whh0": _prep_w(inputs["W_hh0"]),
        "wih0": _prep_w(inputs["W_ih0"]),
        "whh1": _prep_w(inputs["W_hh1"]),
        "wih1": _prep_w(inputs["W_ih1"]),
        "wenc": np.ascontiguousarray(np.asarray(inputs["W_enc"], np.float32).T).astype(BF),
        "wdec": np.ascontiguousarray(np.asarray(inputs["W_dec"], np.float32).T).astype(BF),
        "wout": np.ascontiguousarray(np.asarray(inputs["W_out"], np.float32).T).astype(BF),
        "b0": _prep_b((np.asarray(inputs["b_ih0"], np.float32)
                       + np.asarray(inputs["b_hh0"], np.float32))[_PERM]),
        "b1": _prep_b((np.asarray(inputs["b_ih1"], np.float32)
                       + np.asarray(inputs["b_hh1"], np.float32))[_PERM]),
        "benc": np.ascontiguousarray(
            np.asarray(inputs["b_enc"], np.float32).reshape(4, 128).T),
        "bout": np.ascontiguousarray(
            np.asarray(inputs["b_out"], np.float32).reshape(8, 128).T),
    }
    in_maps = []
    for c in range(NCORES):
        m = dict(common)
        # hsT: [p, ec, (b t)] pre-transposed slice of hs
        hsl = hs_pad[:, c * TLOC:(c + 1) * TLOC, :].reshape(B * TLOC, E)
        m["hst"] = np.ascontiguousarray(
            hsl.T.reshape(4, 128, B * TLOC).transpose(1, 0, 2)).astype(BF)
        in_maps.append(m)
    return in_maps


def _assemble_core_output(o):
    # [ub, oc, hf, p, u4, b, t] -> (B, TLOC, U=ub*8+hf*4+u4, OD=oc*128+p)
    o = np.asarray(o).reshape(NBLK, 8, 2, 128, UBLK // 2, B, TLOC)
    o = np.transpose(o, (5, 6, 0, 2, 4, 1, 3))
    return np.ascontiguousarray(o).reshape(B, TLOC, U, OD).astype(np.float32)


def kernel(**inputs):
    nc = _get_nc()
    in_maps = _make_in_maps(inputs)
    _CACHE["in_maps"] = in_maps
    res = bass_utils.run_bass_kernel_spmd(nc, in_maps, core_ids=list(range(NCORES)))
    outs = [_assemble_core_output(r["out"]) for r in res.results]
    return np.concatenate(outs, axis=1).astype(np.float32)
